# revision 8
# baseline (speedup 1.0000x reference)
"""DIoU regression loss on 8 Trainium2 NeuronCores (data-parallel).

loss = sum(1 - clip(diou(pred_i, gt_i), -1, 1)) / (N + 1e-4) over N=4M boxes.

Sharding: each core gets a contiguous slab of R = 128*T*K rows; the last
core's slab is padded with identical unit boxes whose diou == 1, so padded
rows contribute 0 to sum(1 - diou).

Layout: the host packs 14 bf16 planes per core with w,l,h PRE-HALVED
(w/2, l/2, h/2) so every constant factor in the diou algebra vanishes:
with half-extents e (e_x = (w/2)c + (l/2)s, e_y = (l/2)c - (w/2)s, e_z =
h/2), u_d = ep+eg, v_d = ep-eg, delta = cg-cp, g_d = max(|v_d|,|delta_d|)
(one abs_max ALU op):
  inter_d = relu(u_d - g_d)   (full units),  outer_d = u_d + g_d
so with ti = inter_d/2 (ACT relu scale=0.5), I = prod(ti) = inter_vol/8,
S = volp/8 + volg/8 (from the halved w,l,h), U = S - I = union/8,
O = sum(outer_d^2) = outer_diag, D = sum(delta^2) = center dist^2:
  -diou = (D*U - I*O) / (U*O)   (the 1/8 cancels).
No tensor_scalar constant fixups remain anywhere in the chain, and the
clip to [-1,1] is dropped: 4*D <= O and I <= U hold identically, so
-diou is in [-1,1] up to bf16 rounding.

Engine split: DVE does the binary chain; u/v pair p-with-g via a strided
[P,2,2,T] view of the exy tile (2x mode holds for stride-2 plane views).
ACT does sin/cos, the d2/o2 squares, the in-place ti relu (scale=0.5) and
the reciprocal (AF.Reciprocal) -- the auto-inserted activation-table swaps
(trig <-> reciprocal_and_small) cost ~1.3us each but only ~3 occur per
kernel, and they take the fp32 DVE rcp plus two casts off the critical
engine. The PE row-sums negd for tiles 0..K-2 into PSUM via ones-matmuls;
the last tile's negd is DMA'd out raw in halves and summed by the host.
GPSIMD is unused: its SBUF streaming serializes with every 2-input DVE op
(measured: a concurrent Pool tensor_tensor stretches DVE TT 830ns->3.3us).

sin/cos of tile k+1 are computed during tile k's body (double-buffered)
and the prefetch DMA is deferred mid-body so it never competes with the
current tile's load. Tile 0 orders products before the center deltas so
nothing waits on the centers DMA (last chunk) during the prologue.
"""

import numpy as np
import ml_dtypes

import concourse.bacc as bacc
import concourse.mybir as mybir
import concourse.tile as tile
from concourse import bass_utils

P = 128          # SBUF partitions
T = 1304         # rows per partition per tile
K = 3            # tiles per core
NCORES = 8
RCORE = P * T * K            # 500,736 rows per core
NPAD = RCORE * NCORES        # 4,005,888
NREAL = 4_000_000
C = 14                       # planes
BF16 = mybir.dt.bfloat16
F32 = mybir.dt.float32
HALF_PI = float(np.pi / 2)

AF = mybir.ActivationFunctionType
OP = mybir.AluOpType

# plane order: w_p w_g l_p l_g | r_p r_g | h_p h_g | cp_x cp_y zp cg_x cg_y zg
# (w,l,h pre-halved on the host)
_PLANE_SRC = [(3, 0), (3, 1), (4, 0), (4, 1), (6, 0), (6, 1), (5, 0), (5, 1),
              (0, 0), (1, 0), (2, 0), (0, 1), (1, 1), (2, 1)]
_PLANE_HALF = np.array([1, 1, 1, 1, 0, 0, 1, 1, 0, 0, 0, 0, 0, 0],
                       dtype=np.int32)
_PLANE_PAD = np.array([.5, .5, .5, .5, 0, 0, .5, .5, .3, .3, .3, .3, .3, .3],
                      dtype=np.float32)

_CACHE = {}
_TRACE = False
_LAST = None


def _build():
    nc = bacc.Bacc("TRN2", target_bir_lowering=False, debug=False,
                   num_devices=NCORES)
    ab = nc.dram_tensor("ab", [C, RCORE], BF16, kind="ExternalInput").ap()
    out = nc.dram_tensor("out", [P, 2], F32, kind="ExternalOutput").ap()
    dumpout = nc.dram_tensor("dumpout", [P, 1, T], BF16,
                             kind="ExternalOutput").ap()

    # [k][P, C, T]: plane c of tile k, partition p starts at
    # c*RCORE + k*P*T + p*T
    abv = ab.rearrange("c (k p t) -> k p c t", p=P, t=T)

    with tile.TileContext(nc) as tc:
        with (
            tc.tile_pool(name="raw", bufs=2) as rawp,
            tc.tile_pool(name="tmp", bufs=1) as tmp,
            tc.tile_pool(name="one", bufs=1) as one,
            tc.psum_pool(name="ps", bufs=1) as psp,
        ):
            psum = psp.tile([P, 1], F32, tag="psum", name="psum")
            ones = one.tile([P, 1], BF16, tag="ones", name="ones")
            nc.vector.memset(ones, 1.0)
            halfpi = one.tile([P, 1], F32, tag="halfpi", name="halfpi")
            nc.vector.memset(halfpi, HALF_PI)
            acc = one.tile([P, 2], F32, tag="acc", name="acc")

            # physical buffer tags; later logical tiles reuse earlier tags
            # once the earlier tile is dead (WAR handled by the dep tracker).
            _ALIAS = {
                "sn2": "C0", "cs2": "C1",
                "wc2": "B0", "ls2": "B1", "ws2": "B2", "lc2": "B3",
                "dc3": "A0", "u3": "A1", "v3": "A2", "d23": "A3",
                "g3": "B0", "ti3": "B1", "to3": "B2", "o23": "B3",
                "wl2": "W0", "vol2": "W1",
                "S": "S0", "I": "S1", "D": "S2", "O": "S3", "U": "S4",
                "UO": "S0", "rcpb": "S5", "IO": "A2", "DU": "A1",
                "negnum": "A3", "negd": "S6",
            }
            _BUFS = {"C0": 2, "C1": 2}

            def t(tag, n):
                tt = _ALIAS[tag]
                return tmp.tile([P, n, T], BF16, tag=tt, name=tag,
                                bufs=_BUFS.get(tt, 1))

            def trig(raw_tile):
                sn2 = t("sn2", 2)
                cs2 = t("cs2", 2)
                r2 = raw_tile[:, 4:6]
                nc.scalar.activation(out=sn2, in_=r2, func=AF.Sin)
                # cos(r) = sin(pi/2 - r)
                nc.scalar.activation(out=cs2, in_=r2, func=AF.Sin,
                                     bias=halfpi, scale=-1.0)
                return sn2, cs2

            # raw: planes 0:4 w4 (w_p,w_g,l_p,l_g halved), 4:6 r2,
            #      6:9 cp3, 9:12 cg3
            # h2 (halved) is DMA'd into its own double-buffered tile.
            def dma_tile(k, raw_t, h2_t):
                nc.sync.dma_start(out=raw_t[:, 4:6], in_=abv[k][:, 4:6])
                nc.sync.dma_start(out=raw_t[:, 0:4], in_=abv[k][:, 0:4])
                nc.sync.dma_start(out=h2_t, in_=abv[k][:, 6:8])
                nc.sync.dma_start(out=raw_t[:, 6:12], in_=abv[k][:, 8:14])

            # prologue: tile 0 DMA (r planes first so trig starts early)
            raw = rawp.tile([P, 12, T], BF16, tag="raw", name="raw")
            h2 = rawp.tile([P, 2, T], BF16, tag="h2", name="h2")
            dma_tile(0, raw, h2)
            sn2, cs2 = trig(raw)

            for k in range(K):
                last = k == K - 1
                raw_nx = h2_nx = None

                w2 = raw[:, 0:2]
                l2 = raw[:, 2:4]

                # --- rotated half-extents (VEC); sin-consumers first so
                # the products start as soon as sin lands ---
                ls2 = t("ls2", 2)
                ws2 = t("ws2", 2)
                wc2 = t("wc2", 2)
                lc2 = t("lc2", 2)
                nc.vector.tensor_mul(ls2, l2, sn2)
                nc.vector.tensor_mul(ws2, w2, sn2)
                nc.vector.tensor_mul(wc2, w2, cs2)
                nc.vector.tensor_mul(lc2, l2, cs2)

                dc3 = t("dc3", 3)
                d23 = t("d23", 3)

                def emit_dc():
                    # center deltas; ACT squares them (D pieces) then takes
                    # |dc| in place (g needs it; abs_max TT fails codegen)
                    nc.vector.tensor_sub(dc3, raw[:, 9:12], raw[:, 6:9])
                    nc.scalar.activation(out=d23, in_=dc3, func=AF.Square)
                    nc.scalar.activation(out=dc3, in_=dc3, func=AF.Abs)

                if k > 0:
                    emit_dc()

                # --- assemble (Ex_p,Ex_g,Ey_p,Ey_g) then u/v via the
                # [P,2,2,T] strided view; z comes from h2 ---
                exy = tmp.tile([P, 4, T], BF16, tag="E4", name="exy")
                nc.vector.tensor_add(exy[:, 0:2], wc2, ls2)
                nc.vector.tensor_sub(exy[:, 2:4], lc2, ws2)
                exyv = exy.rearrange("p (d b) t -> p d b t", d=2, b=2)
                u3 = t("u3", 3)
                v3 = t("v3", 3)
                nc.vector.tensor_add(u3[:, 0:2], exyv[:, :, 0],
                                     exyv[:, :, 1])
                nc.vector.tensor_sub(v3[:, 0:2], exyv[:, :, 0],
                                     exyv[:, :, 1])
                nc.vector.tensor_add(u3[:, 2:3], h2[:, 0:1], h2[:, 1:2])
                nc.vector.tensor_sub(v3[:, 2:3], h2[:, 0:1], h2[:, 1:2])

                if k == 0:
                    # tile 0: centers are the last DMA chunk; deltas only now
                    emit_dc()

                # --- g = max(|v|, |delta|) ---
                nc.scalar.activation(out=v3, in_=v3, func=AF.Abs)
                g3 = t("g3", 3)
                nc.vector.tensor_tensor(out=g3, in0=v3, in1=dc3,
                                        op=OP.max)

                # --- inter/outer ---
                ti3 = t("ti3", 3)
                to3 = t("to3", 3)
                nc.vector.tensor_sub(ti3, u3, g3)
                nc.vector.tensor_add(to3, u3, g3)
                # only outer_y can be negative
                nc.vector.tensor_scalar_max(to3[:, 1:2], to3[:, 1:2], 0.0)
                # ti -> inter_d/2 so I = inter_vol/8 matches U = union/8
                nc.scalar.activation(out=ti3, in_=ti3, func=AF.Relu,
                                     scale=0.5)
                o23 = t("o23", 3)
                nc.scalar.activation(out=o23, in_=to3, func=AF.Square)

                # --- volumes (VEC filler while ACT squares/relus land) ---
                wl2 = t("wl2", 2)
                vol2 = t("vol2", 2)
                S = t("S", 1)
                nc.vector.tensor_mul(wl2, w2, l2)
                nc.vector.tensor_mul(vol2, wl2, h2)
                nc.vector.tensor_add(S, vol2[:, 0:1], vol2[:, 1:2])

                # deferred prefetch: next tile's DMA only now so it never
                # steals HBM bandwidth from the current tile's load
                if k + 1 < K:
                    raw_nx = rawp.tile([P, 12, T], BF16, tag="raw",
                                       name="raw")
                    h2_nx = rawp.tile([P, 2, T], BF16, tag="h2", name="h2")
                    dma_tile(k + 1, raw_nx, h2_nx)

                # --- plane reductions ---
                I = t("I", 1)
                nc.vector.tensor_mul(I, ti3[:, 0:1], ti3[:, 1:2])
                nc.vector.tensor_mul(I, I, ti3[:, 2:3])
                D = t("D", 1)
                nc.vector.tensor_add(D, d23[:, 0:1], d23[:, 1:2])
                nc.vector.tensor_add(D, D, d23[:, 2:3])
                O = t("O", 1)
                nc.vector.tensor_add(O, o23[:, 0:1], o23[:, 1:2])
                nc.vector.tensor_add(O, O, o23[:, 2:3])

                # next tile's trig: after the small-table ACT ops so the
                # table sequence stays [small] -> [trig] -> [recip]
                if raw_nx is not None:
                    sn_nx, cs_nx = trig(raw_nx)

                # --- -diou = (D*U - I*O) / (U*O) ---
                U = t("U", 1)
                nc.vector.tensor_sub(U, S, I)
                UO = t("UO", 1)
                nc.vector.tensor_mul(UO, U, O)
                # fp32 reciprocal on the DVE (ACT Reciprocal is blocked by
                # bass for accuracy); ACT does the casts around it
                UOf = tmp.tile([P, 1, T], F32, tag="F0", name="UOf")
                nc.scalar.copy(UOf, UO)
                rcpf = tmp.tile([P, 1, T], F32, tag="F1", name="rcpf")
                nc.vector.reciprocal_approx_fast(out=rcpf, in_=UOf)
                rcpb = t("rcpb", 1)
                nc.scalar.copy(rcpb, rcpf)
                IO = t("IO", 1)
                DU = t("DU", 1)
                nc.vector.tensor_mul(IO, I, O)
                nc.vector.tensor_mul(DU, D, U)
                negnum = t("negnum", 1)
                nc.vector.tensor_sub(negnum, DU, IO)
                # clip to [-1,1]: mathematically a no-op, but it sanitizes
                # the inf/NaN a bf16-cancelled U (S ~ I for near-identical
                # boxes) pushes through the reciprocal
                negd = t("negd", 1)

                def mul_clip(sl):
                    nc.vector.tensor_mul(negd[:, :, sl], negnum[:, :, sl],
                                         rcpb[:, :, sl])
                    nc.vector.tensor_scalar(out=negd[:, :, sl],
                                            in0=negd[:, :, sl], scalar1=1.0,
                                            scalar2=-1.0, op0=OP.min,
                                            op1=OP.max)

                if last:
                    # split so the first half's output DMA overlaps the
                    # second half's multiply
                    H = T // 2
                    mul_clip(slice(0, H))
                    nc.sync.dma_start(out=dumpout[:, :, 0:H],
                                      in_=negd[:, :, 0:H])
                    mul_clip(slice(H, T))
                    nc.sync.dma_start(out=dumpout[:, :, H:T],
                                      in_=negd[:, :, H:T])
                else:
                    mul_clip(slice(0, T))
                    # row-sum on the otherwise idle PE
                    for j in range((T + P - 1) // P):
                        lo, hi = j * P, min((j + 1) * P, T)
                        nc.tensor.matmul(psum[0:hi - lo], negd[:, 0, lo:hi],
                                         ones, start=(k == 0 and j == 0),
                                         stop=(k == K - 2 and hi == T))
                    if k == K - 2:
                        # drain PSUM during the final tile's compute
                        nc.scalar.copy(acc[:, 0:1], psum)
                        nc.sync.dma_start(out=out[:, 0:1], in_=acc[:, 0:1])

                if raw_nx is not None:
                    raw, h2, sn2, cs2 = raw_nx, h2_nx, sn_nx, cs_nx

    nc.compile()
    return nc


def _pack_planes(box_pred, box_gt):
    """Full [C, NPAD] bf16 plane-major array (w,l,h halved)."""
    planes = np.empty((C, NPAD), dtype=ml_dtypes.bfloat16)
    src = (box_pred, box_gt)
    for i, (col, which) in enumerate(_PLANE_SRC):
        col_v = src[which][:, col]
        if _PLANE_HALF[i]:
            col_v = col_v * np.float32(0.5)
        planes[i, :NREAL] = col_v.astype(ml_dtypes.bfloat16)
        planes[i, NREAL:] = _PLANE_PAD[i]
    return planes


def kernel(box_pred, box_gt):
    global _LAST
    box_pred = np.asarray(box_pred, dtype=np.float32)
    box_gt = np.asarray(box_gt, dtype=np.float32)
    n = box_pred.shape[0]
    assert n == NREAL, f"kernel hardcoded for N={NREAL}, got {n}"

    if "nc" not in _CACHE:
        _CACHE["nc"] = _build()
    nc = _CACHE["nc"]

    planes = _pack_planes(box_pred, box_gt)
    in_maps = []
    for c in range(NCORES):
        lo, hi = c * RCORE, (c + 1) * RCORE
        in_maps.append({"ab": np.ascontiguousarray(planes[:, lo:hi])})

    kw = dict(trace=True, trace_cores=[0]) if _TRACE else {}
    res = bass_utils.run_bass_kernel_spmd(nc, in_maps,
                                          core_ids=list(range(NCORES)), **kw)
    _LAST = res
    total_neg = sum(
        float(res.results[c]["out"][:, 0].astype(np.float64).sum())
        + float(res.results[c]["dumpout"].astype(np.float64).sum())
        for c in range(NCORES)
    )
    loss = (NPAD + total_neg) / (NREAL + 1e-4)
    return np.float32(loss)
